# revision 1
# baseline (speedup 1.0000x reference)
"""Trainium2 Bass kernel for the gnn_message_passing problem.

Math (per edge e, side i):
  node_feat = l2norm(|dt|*w_time + b_time + gc*w_node + b_node)
  neigh_feat likewise per neighbor k
  att = tanh(node_feat@Wq + neigh_feat@Wk) . v_att
  score = leaky_relu(att + 2/(2+dt_neigh), 0.01)
  agg = sum_k (score*mask/n_neigh) * neigh_feat
  combined = [node_feat, agg]
  feat = sum_w exp(-0.5*bank_dt)*bank_mask * bank_feat + combined
  out = relu(feat @ weight.T)

Key structure exploited: every featurized vector lies in span{w_time, w_node,
b_time+b_node}, so node/neigh features are 3 scalars each. q+kk collapses to a
rank-6 combination of 6 fixed D-vectors; the "combined @ W.T" part of the
output collapses to a rank-6 combination of 6 fixed H-vectors. Only the
tanh( . ) . v contraction (E*2*K*D tanh evals) and the bank-feature reduction
touch O(E*K*D)-sized data on-device.

Sharding: pure data-parallel over E across 8 cores (one SPMD program).
"""

import numpy as np
import ml_dtypes

import concourse.bass as bass
import concourse.bacc as bacc
import concourse.mybir as mybir
import concourse.tile as tile
from concourse.bass_utils import run_bass_kernel_spmd

F32 = mybir.dt.float32
BF16 = mybir.dt.bfloat16
AF = mybir.ActivationFunctionType
OP = mybir.AluOpType

E, K, W, D, H = 4096, 32, 8, 128, 256
NCORES = 8
EC = E // NCORES          # 512 edges per core
POS = EC * 2              # 1024 (edge, side) positions per core
NT = POS // 128           # 8 position tiles of 128
D2 = 2 * D                # 256
CHUNKS = 4                # tanh chunks of 1024 cols per tile
VLAG = 2                  # vdot trails arg/tanh by 2 chunks


def _build_program(pp):
    """Build the SPMD single-core program. pp: dict of host-precomputed params."""
    nc = bacc.Bacc("TRN2", target_bir_lowering=False, debug=False)

    # ---- DRAM I/O (per core shard), host-prepermuted layouts ----
    d_dtn = nc.dram_tensor("dtn_p", [128, 256], F32, kind="ExternalInput")
    d_gcn = nc.dram_tensor("gcn_p", [128, 256], F32, kind="ExternalInput")
    d_msk = nc.dram_tensor("mskn_p", [128, 256], F32, kind="ExternalInput")
    d_dts = nc.dram_tensor("dts_p", [128, 8], F32, kind="ExternalInput")
    d_gcs = nc.dram_tensor("gcs_p", [128, 8], F32, kind="ExternalInput")
    d_bdt = nc.dram_tensor("bdt_e", [128, 64], F32, kind="ExternalInput")
    d_bmsk = nc.dram_tensor("bmsk_e", [128, 64], F32, kind="ExternalInput")
    # chunk-contiguous bf16: chunk c=(t*4+j)*2+wh -> rows c*128..(c+1)*128
    d_bft = nc.dram_tensor("bft_p", [64 * 128, D2], F32, kind="ExternalInput")
    d_out = nc.dram_tensor("out", [POS, H], F32, kind="ExternalOutput")

    # ---- inline constants ----
    c_basis = nc.inline_tensor(pp["basis6att"], name="c_basis")    # [6,128] bf16
    c_b6h = nc.inline_tensor(pp["basis6H"], name="c_b6h")          # [6,256] f32
    c_v = nc.inline_tensor(pp["v32"], name="c_v")                  # [128,32] f32
    c_wT = nc.inline_tensor(pp["weightT"], name="c_wT")            # [256,256] f32
    c_dmask = nc.inline_tensor(pp["dmask"], name="c_dmask")        # [128,32] f32
    c_ident = nc.inline_tensor(pp["ident"], name="c_ident")        # [128,128] f32
    G = pp["gram"]  # 3x3 float

    from contextlib import ExitStack
    with tile.TileContext(nc) as tc, ExitStack() as ctx:
        cpool = ctx.enter_context(tc.tile_pool(name="consts", bufs=1))
        wpool = ctx.enter_context(tc.tile_pool(name="work", bufs=1))
        p_coef6 = ctx.enter_context(tc.tile_pool(name="coef6", bufs=2))
        p_tanh = ctx.enter_context(tc.tile_pool(name="tanh", bufs=6))
        p_attT = ctx.enter_context(tc.tile_pool(name="attT", bufs=3))
        p_featT = ctx.enter_context(tc.tile_pool(name="featT", bufs=4))
        p_bch = ctx.enter_context(tc.tile_pool(name="bch", bufs=10))
        p_mblk = ctx.enter_context(tc.tile_pool(name="mblk", bufs=2))
        p_bankC = ctx.enter_context(tc.tile_pool(name="bankC", bufs=8))
        p_out = ctx.enter_context(tc.tile_pool(name="outp", bufs=2))
        ps_arg = ctx.enter_context(tc.tile_pool(name="ps_arg", bufs=2, space="PSUM"))
        ps_mix = ctx.enter_context(tc.tile_pool(name="ps_mix", bufs=4, space="PSUM"))

        # ---- loads ----
        bdt_e = wpool.tile([128, 64], F32, name="bdt_e")
        bmsk_e = wpool.tile([128, 64], F32, name="bmsk_e")
        nc.sync.dma_start(out=bdt_e, in_=d_bdt[:, :])
        nc.sync.dma_start(out=bmsk_e, in_=d_bmsk[:, :])
        t_dtn = wpool.tile([128, 256], F32, name="t_dtn")
        nc.sync.dma_start(out=t_dtn[:, :], in_=d_dtn[:, :])
        t_m = wpool.tile([128, 256], F32, name="t_m")
        nc.sync.dma_start(out=t_m[:, :], in_=d_msk[:, :])
        a_all = wpool.tile([128, 264], F32, name="a_all")
        b_all = wpool.tile([128, 264], F32, name="b_all")
        nc.sync.dma_start(out=a_all[:, 0:256], in_=d_dtn[:, :])
        nc.sync.dma_start(out=a_all[:, 256:264], in_=d_dts[:, :])
        nc.sync.dma_start(out=b_all[:, 0:256], in_=d_gcn[:, :])
        nc.sync.dma_start(out=b_all[:, 256:264], in_=d_gcs[:, :])

        # ---- constants to SBUF ----
        cb_basis = cpool.tile([6, 128], BF16, name="cb_basis")
        nc.scalar.dma_start(out=cb_basis, in_=c_basis[:, :])
        cb_b6h = cpool.tile([6, 256], F32, name="cb_b6h")
        nc.scalar.dma_start(out=cb_b6h, in_=c_b6h[:, :])
        cb_v = cpool.tile([128, 32], F32, name="cb_v")
        nc.scalar.dma_start(out=cb_v, in_=c_v[:, :])
        cb_wT0 = cpool.tile([128, 256], F32, name="cb_wT0")
        nc.scalar.dma_start(out=cb_wT0, in_=c_wT[0:128, :])
        cb_wT1 = cpool.tile([128, 256], F32, name="cb_wT1")
        nc.scalar.dma_start(out=cb_wT1, in_=c_wT[128:256, :])
        cb_dmask = cpool.tile([128, 32], F32, name="cb_dmask")
        nc.scalar.dma_start(out=cb_dmask, in_=c_dmask[:, :])
        cb_id = cpool.tile([128, 128], F32, name="cb_id")
        nc.scalar.dma_start(out=cb_id, in_=c_ident[:, :])

        # ---- bank decay weights first (ACT exp before sqrt: unblocks bank
        # pipeline; costs one extra table load, hidden early) ----
        bwe = wpool.tile([128, 64], F32, name="bwe")
        nc.scalar.activation(out=bwe, in_=bdt_e, func=AF.Exp, scale=-0.5)
        nc.vector.tensor_tensor(out=bwe, in0=bwe, in1=bmsk_e, op=OP.mult)

        # ---- featurize scalars ----
        nega = wpool.tile([128, 264], F32, name="nega")
        nc.vector.tensor_scalar(out=nega, in0=a_all, scalar1=-1.0, scalar2=None,
                                op0=OP.mult)
        nc.vector.tensor_tensor(out=a_all, in0=a_all, in1=nega, op=OP.max)
        aa = wpool.tile([128, 264], F32, name="aa")
        ab = wpool.tile([128, 264], F32, name="ab")
        bb = wpool.tile([128, 264], F32, name="bb")
        nc.vector.tensor_tensor(out=aa, in0=a_all, in1=a_all, op=OP.mult)
        nc.vector.tensor_tensor(out=ab, in0=a_all, in1=b_all, op=OP.mult)
        nc.vector.tensor_tensor(out=bb, in0=b_all, in1=b_all, op=OP.mult)
        n2 = wpool.tile([128, 264], F32, name="n2")
        nc.vector.tensor_scalar(out=n2, in0=aa, scalar1=float(G[0, 0]),
                                scalar2=float(G[2, 2]), op0=OP.mult, op1=OP.add)
        nc.vector.scalar_tensor_tensor(out=n2, in0=bb, scalar=float(G[1, 1]),
                                       in1=n2, op0=OP.mult, op1=OP.add)
        nc.vector.scalar_tensor_tensor(out=n2, in0=a_all, scalar=float(2 * G[0, 2]),
                                       in1=n2, op0=OP.mult, op1=OP.add)
        nc.vector.scalar_tensor_tensor(out=n2, in0=b_all, scalar=float(2 * G[1, 2]),
                                       in1=n2, op0=OP.mult, op1=OP.add)
        nc.vector.scalar_tensor_tensor(out=n2, in0=ab, scalar=float(2 * G[0, 1]),
                                       in1=n2, op0=OP.mult, op1=OP.add)
        nrm = wpool.tile([128, 264], F32, name="nrm")
        nc.scalar.activation(out=nrm, in_=n2, func=AF.Sqrt)
        nc.vector.tensor_scalar(out=nrm, in0=nrm, scalar1=1e-12, scalar2=None,
                                op0=OP.max)
        scr = wpool.tile([128, 264], F32, name="scr")
        invn = wpool.tile([128, 264], F32, name="invn")
        nc.vector.reciprocal_approx_accurate(out=invn, in_=nrm, scratch=scr)
        alpha = wpool.tile([128, 264], F32, name="alpha")
        beta = wpool.tile([128, 264], F32, name="beta")
        nc.vector.tensor_tensor(out=alpha, in0=a_all, in1=invn, op=OP.mult)
        nc.vector.tensor_tensor(out=beta, in0=b_all, in1=invn, op=OP.mult)

        # time decay 2/(2+dt) on raw dt
        ts_t = wpool.tile([128, 256], F32, name="ts_t")
        scr2 = wpool.tile([128, 256], F32, name="scr2")
        nc.vector.tensor_scalar(out=ts_t, in0=t_dtn, scalar1=2.0, scalar2=None,
                                op0=OP.add)
        nc.vector.reciprocal_approx_accurate(out=ts_t, in_=ts_t, scratch=scr2)
        nc.vector.tensor_scalar(out=ts_t, in0=ts_t, scalar1=2.0, scalar2=None,
                                op0=OP.mult)

        # n_neigh and mask/n_neigh
        nn = wpool.tile([128, 8], F32, name="nn")
        nc.vector.tensor_reduce(out=nn, in_=t_m.rearrange("p (t k) -> p t k", k=K),
                                axis=mybir.AxisListType.X, op=OP.add)
        nc.vector.tensor_scalar(out=nn, in0=nn, scalar1=1.0, scalar2=None,
                                op0=OP.max)
        innn = wpool.tile([128, 8], F32, name="innn")
        scr3 = wpool.tile([128, 8], F32, name="scr3")
        nc.vector.reciprocal_approx_accurate(out=innn, in_=nn, scratch=scr3)
        mrec = wpool.tile([128, 256], F32, name="mrec")
        nc.vector.tensor_tensor(
            out=mrec.rearrange("p (t k) -> p t k", k=K),
            in0=t_m.rearrange("p (t k) -> p t k", k=K),
            in1=innn.unsqueeze(2).broadcast_to([128, 8, K]), op=OP.mult)

        att_a = wpool.tile([128, 256], F32, name="att_a")
        coefF6 = wpool.tile([6, 8 * 128], F32, name="coefF6")
        ABC = wpool.tile([128, 24], F32, name="ABC")  # cols c*8+t
        bankC_sb = [None] * NT

        # ---- helpers ----
        def build_coef6(t):
            c6 = p_coef6.tile([6, 4096], BF16, tag="coef6", name=f"coef6_{t}")
            for c in range(3):
                r = 3 * t + c
                nc.sync.dma_start(
                    out=c6[c:c + 1, :],
                    in_=selfT[r:r + 1, :].unsqueeze(1).broadcast_to(
                        [1, K, 128]))
            ch = coefT_h[t // 4]
            for c in range(3):
                eng = nc.gpsimd if c % 2 else nc.sync
                eng.dma_start(
                    out=c6[3 + c:4 + c, :],
                    in_=ch[(t % 4) * 32:(t % 4) * 32 + 32,
                           c * 128:(c + 1) * 128])
            return c6

        def build_mb(t):
            mb = p_mblk.tile([128, 256], F32, tag="mblk", name=f"mb_{t}")
            nc.vector.tensor_tensor(
                out=mb.rearrange("r (b c) -> r b c", c=32),
                in0=cb_dmask.unsqueeze(1).broadcast_to([128, 8, 32]),
                in1=bwe[:, t * 8:(t + 1) * 8].unsqueeze(2).broadcast_to(
                    [128, 8, 32]),
                op=OP.mult)
            return mb

        def load_bc(gidx):
            bc = p_bch.tile([128, 256], F32, tag="bch", name=f"bc_{gidx}")
            eng = nc.gpsimd if gidx % 2 else nc.sync
            eng.dma_start(out=bc[:, :],
                          in_=d_bft[gidx * 128:(gidx + 1) * 128, :])
            return bc

        bc_pend = []

        def bank_open(tb):
            return {"mb": build_mb(tb),
                    "fpA": ps_mix.tile([128, 512], F32, tag="mix",
                                       name=f"fpA_{tb}")}

        def bank_tile_mms(tb, bst):
            # 8 chunks for this tile already loaded in bc_pend[0:8]
            bcs = [bc_pend.pop(0) for _ in range(8)]
            for wh in range(2):
                for j in range(4):      # 4 col-groups back-to-back: overlap
                    nc.tensor.matmul(
                        bst["fpA"][32 * j:32 * (j + 1), 0:256],
                        lhsT=bst["mb"][:, 32 * (2 * j + wh):
                                       32 * (2 * j + wh + 1)],
                        rhs=bcs[2 * j + wh][:, :],
                        start=(wh == 0), stop=(wh == 1),
                        skip_group_check=True,
                        tile_position=(0, 32 * j))

        def bank_close(tb, bst):
            bkA = p_mblk.tile([128, 256], F32, tag="bkA", name=f"bkA_{tb}")
            nc.vector.tensor_copy(out=bkA, in_=bst["fpA"][:, 0:256])
            fsb = [None, None]
            for h in range(2):
                pmb = ps_mix.tile([128, 512], F32, tag="mix",
                                  name=f"pmb_{tb}_{h}")
                nc.tensor.transpose(pmb[0:128, 0:128],
                                    bkA[:, h * 128:(h + 1) * 128], cb_id)
                fsb[h] = p_featT.tile([128, 128], F32, tag="featT",
                                      name=f"fT_{tb}_{h}")
                nc.vector.tensor_copy(out=fsb[h], in_=pmb[0:128, 0:128])
            poB = ps_mix.tile([128, 512], F32, tag="mix", name=f"poB_{tb}")
            nc.tensor.matmul(poB[:, 0:256], lhsT=fsb[0], rhs=cb_wT0,
                             start=True, stop=False)
            nc.tensor.matmul(poB[:, 0:256], lhsT=fsb[1], rhs=cb_wT1,
                             start=False, stop=True)
            bankC_sb[tb] = p_bankC.tile([128, 256], F32, tag="bankC",
                                        name=f"bankC_{tb}")
            nc.vector.tensor_copy(out=bankC_sb[tb], in_=poB[:, 0:256])

        # prologue: tiles 0..3 bank processing fills PE while the DVE scalar
        # chain computes the attention coefficients
        for g in range(8):
            bc_pend.append(load_bc(g))
        for tb in range(2):
            for g in range(8):
                if (tb + 1) * 8 + g < 64:
                    bc_pend.append(load_bc((tb + 1) * 8 + g))
            bst = bank_open(tb)
            bank_tile_mms(tb, bst)
            bank_close(tb, bst)

        # ---- transposes for coef rows ----
        packS = wpool.tile([128, 24], F32, name="packS")
        nc.vector.tensor_copy(
            out=packS.rearrange("p (t c) -> p t c", c=3)[:, :, 0],
            in_=alpha[:, 256:264])
        nc.vector.tensor_copy(
            out=packS.rearrange("p (t c) -> p t c", c=3)[:, :, 1],
            in_=beta[:, 256:264])
        nc.vector.tensor_copy(
            out=packS.rearrange("p (t c) -> p t c", c=3)[:, :, 2],
            in_=invn[:, 256:264])
        pm = ps_mix.tile([128, 512], F32, tag="mix", name="pm_selfT")
        nc.tensor.transpose(pm[0:24, 0:128], packS, cb_id)
        selfT = wpool.tile([32, 128], BF16, name="selfT")
        nc.vector.tensor_copy(out=selfT[0:24, :], in_=pm[0:24, 0:128])

        coefT_h = [wpool.tile([128, 384], BF16, name=f"coefTh{h}")
                   for h in range(2)]
        for ci, srcT in enumerate((alpha, beta, invn)):
            for h in range(2):
                pmx = ps_mix.tile([128, 512], F32, tag="mix",
                                  name=f"pm_{ci}{h}")
                nc.tensor.transpose(pmx[0:128, 0:128],
                                    srcT[:, h * 128:(h + 1) * 128], cb_id)
                nc.vector.tensor_copy(
                    out=coefT_h[h][:, ci * 128:(ci + 1) * 128],
                    in_=pmx[0:128, 0:128])

        coef6_t = build_coef6(0)
        state = {}
        pend = []               # [(th, cc, t)] vdots not yet emitted
        pend_pmxa = []          # att transposes delayed one chunk
        pv_by_group = {}

        def flush_pmxa():
            while pend_pmxa:
                tx = pend_pmxa.pop(0)
                attT = state[tx]["attT"]
                pmx = ps_mix.tile([128, 512], F32, tag="mix", name=f"pmxa_{tx}")
                nc.tensor.transpose(pmx[0:128, 0:32], attT, cb_id[0:32, 0:32])
                nc.vector.tensor_copy(out=att_a[:, 32 * tx:32 * (tx + 1)],
                                      in_=pmx[0:128, 0:32])
                if tx == 3:
                    emit_score_half(0)

        def emit_vgroup(th0, th1, cc1, t):
            # 4 col-group matmuls back-to-back: concurrent in the PE array
            g = (t * CHUNKS + cc1) // 2
            pv = ps_mix.tile([128, 512], F32, tag="mix", name=f"pv_{g}")
            for q, (thx, mm) in enumerate(((th0, 0), (th0, 1),
                                           (th1, 0), (th1, 1))):
                nc.tensor.matmul(pv[32 * q:32 * (q + 1), :], lhsT=cb_v,
                                 rhs=thx[:, mm * 512:(mm + 1) * 512],
                                 start=True, stop=True,
                                 tile_position=(0, 32 * q))
            b = cc1 // 2
            ast = p_mblk.tile([128, 512], F32, tag="astage",
                              name=f"ast_{t}_{cc1}")
            nc.vector.tensor_copy(out=ast[:, :], in_=pv[:, :])
            attT = state[t]["attT"]
            nc.sync.dma_start(
                out=attT[16 * b:16 * (b + 1), :],
                in_=ast.rearrange("(q r) (kl p) -> q r kl p",
                                  r=32, p=128)[:, 0])
            if cc1 == 3:
                pend_pmxa.append(t)

        sc = wpool.tile([128, 256], F32, name="sc")
        sc2 = wpool.tile([128, 256], F32, name="sc2")
        wgt = wpool.tile([128, 256], F32, name="wgt")
        prod = wpool.tile([128, 256], F32, name="prod")

        def emit_score_half(hh):
            s = slice(hh * 128, (hh + 1) * 128)
            nc.vector.tensor_tensor(out=sc[:, s], in0=att_a[:, s],
                                    in1=ts_t[:, s], op=OP.add)
            nc.vector.tensor_scalar(out=sc2[:, s], in0=sc[:, s], scalar1=0.01,
                                    scalar2=None, op0=OP.mult)
            nc.vector.tensor_tensor(out=sc[:, s], in0=sc[:, s], in1=sc2[:, s],
                                    op=OP.max)
            nc.vector.tensor_tensor(out=wgt[:, s], in0=sc[:, s],
                                    in1=mrec[:, s], op=OP.mult)
            for c, csrc in enumerate((alpha, beta, invn)):
                nc.vector.tensor_tensor(out=prod[:, s], in0=wgt[:, s],
                                        in1=csrc[:, s], op=OP.mult)
                nc.vector.tensor_reduce(
                    out=ABC[:, c * 8 + hh * 4:c * 8 + (hh + 1) * 4],
                    in_=prod[:, s].rearrange("p (t k) -> p t k", k=K),
                    axis=mybir.AxisListType.X, op=OP.add)

        # ---- software-pipelined global chunk loop ----
        bst = None
        for gc in range(NT * CHUNKS):
            t, cc = divmod(gc, CHUNKS)
            tb = t + 2           # bank tile handled during this att tile
            if cc == 0:
                state[t] = {
                    "attT": p_attT.tile([32, 128], F32, tag="attT",
                                        name=f"attT_{t}"),
                    "coef6": coef6_t,
                }
                if t + 1 < NT:
                    coef6_t = build_coef6(t + 1)
                if tb < NT:
                    for g in range(8):
                        if (tb + 1) * 8 + g < 64:
                            bc_pend.append(load_bc((tb + 1) * 8 + g))
                    bst = bank_open(tb)
                    bank_tile_mms(tb, bst)
                    bank_close(tb, bst)
            st = state[t]
            pa = ps_arg.tile([128, 1024], F32, tag="psarg", name=f"pa_{gc}")
            for mm in range(2):
                nc.tensor.matmul(
                    pa[:, mm * 512:(mm + 1) * 512], lhsT=cb_basis,
                    rhs=st["coef6"][:, cc * 1024 + mm * 512:
                                    cc * 1024 + (mm + 1) * 512],
                    start=True, stop=True)
            th = p_tanh.tile([128, 1024], F32, tag="tanh", name=f"th_{gc}")
            nc.scalar.activation(out=th, in_=pa, func=AF.Tanh)
            if len(pend) >= 4 and pend[0][1] % 2 == 0:
                (th0, _, _), (th1, cc1, t1) = pend.pop(0), pend.pop(0)
                emit_vgroup(th0, th1, cc1, t1)
            pend.append((th, cc, t))
            flush_pmxa()
        while pend:
            (th0, _, _), (th1, cc1, t1) = pend.pop(0), pend.pop(0)
            emit_vgroup(th0, th1, cc1, t1)
            flush_pmxa()

        # ---- score + agg coefficients: second half (0:128 emitted mid-loop) ----
        emit_score_half(1)

        # pack final rank-6 coefs: col = c*8 + t, rows: (as,bs,gs,A,B,C)
        packF = wpool.tile([128, 48], F32, name="packF")
        for c, src in ((0, alpha[:, 256:264]), (1, beta[:, 256:264]),
                       (2, invn[:, 256:264]), (3, ABC[:, 0:8]),
                       (4, ABC[:, 8:16]), (5, ABC[:, 16:24])):
            nc.vector.tensor_copy(out=packF[:, c * 8:(c + 1) * 8], in_=src)
        pmf = ps_mix.tile([128, 512], F32, tag="mix", name="pm_packF")
        nc.tensor.transpose(pmf[0:48, 0:128], packF, cb_id)
        pFT = wpool.tile([48, 128], F32, name="pFT")
        nc.vector.tensor_copy(out=pFT, in_=pmf[0:48, 0:128])
        for c in range(6):
            eng = nc.gpsimd if c % 2 else nc.sync
            eng.dma_start(out=coefF6[c:c + 1, :],
                          in_=pFT[c * 8:(c + 1) * 8, :])

        # ---- tail: rank-6 combined part + add + relu + store ----
        for t in range(NT):
            pc = ps_mix.tile([128, 512], F32, tag="mix", name=f"pc_{t}")
            nc.tensor.matmul(pc[:, 0:256], lhsT=coefF6[:, t * 128:(t + 1) * 128],
                             rhs=cb_b6h, start=True, stop=True)
            ot = p_out.tile([128, 256], F32, tag="outp", name=f"ot_{t}")
            nc.vector.tensor_tensor(out=ot, in0=pc[:, 0:256], in1=bankC_sb[t],
                                    op=OP.add)
            nc.scalar.activation(out=ot, in_=ot, func=AF.Relu)
            nc.gpsimd.dma_start(out=d_out[t * 128:(t + 1) * 128, :], in_=ot)

    nc.compile()
    return nc


def _host_params(w_time, b_time, w_node, b_node, Wq, Wk, v_att, weight):
    f32 = np.float32
    w_time = np.asarray(w_time, f32)
    w_node = np.asarray(w_node, f32)
    bsum = np.asarray(b_time, f32) + np.asarray(b_node, f32)
    Wq = np.asarray(Wq, f32)
    Wk = np.asarray(Wk, f32)
    v = np.asarray(v_att, f32)
    weight = np.asarray(weight, f32)

    basis3 = np.stack([w_time, w_node, bsum])                  # [3, D]
    gram = basis3 @ basis3.T
    basis6att = np.zeros((6, D), f32)
    basis6att[0:3] = basis3 @ Wq
    basis6att[3:6] = basis3 @ Wk
    basis6H = np.zeros((6, H), f32)
    basis6H[0:3] = basis3 @ weight[:, :D].T
    basis6H[3:6] = basis3 @ weight[:, D:].T
    dmask = np.zeros((128, 32), f32)
    dmask[np.arange(128), np.arange(128) // 4] = 1.0
    return {
        "basis6att": basis6att.astype(ml_dtypes.bfloat16),
        "basis6H": basis6H,
        "v32": np.ascontiguousarray(np.tile(v.reshape(D, 1), (1, 32))),
        "weightT": np.ascontiguousarray(weight.T),
        "dmask": dmask,
        "ident": np.eye(128, dtype=f32),
        "gram": gram.astype(np.float64),
    }


def _perm_tk(x):
    # [EC,2,K] -> [128 p, (t k)]
    return np.ascontiguousarray(
        x.reshape(NT, 128, K).transpose(1, 0, 2).reshape(128, NT * K))


def _perm_t(x):
    # [EC,2] -> [128 p, t]
    return np.ascontiguousarray(x.reshape(NT, 128).T)


def _perm_bft(x):
    # [EC,2,W,D2] -> rows ((t j wh),(po wl)) x D2, bf16
    x = x.reshape(NT, 4, 32, 2, 4, D2)       # t j po wh wl d
    x = x.transpose(0, 1, 3, 2, 4, 5)        # t j wh po wl d
    return np.ascontiguousarray(x.reshape(64 * 128, D2))


def _expand_bank(x):
    # [EC,2,W] -> [128 (po,wl), 64 (t,j,wh)]: x[t*128+j*32+po, wh*4+wl]
    x = x.reshape(NT, 4, 32, 2, 4)          # t j po wh wl
    x = x.transpose(2, 4, 0, 1, 3)          # po wl t j wh
    return np.ascontiguousarray(x.reshape(128, 64))


def _shard_inputs(inputs):
    f32 = np.float32
    ins = []
    for c in range(NCORES):
        sl = slice(c * EC, (c + 1) * EC)
        ins.append({
            "dtn_p": _perm_tk(np.asarray(inputs["dt_neigh"][sl], f32)),
            "gcn_p": _perm_tk(np.asarray(inputs["gc_neigh"][sl], f32)),
            "mskn_p": _perm_tk(
                np.asarray(inputs["neigh_mask"][sl]).astype(f32)),
            "dts_p": _perm_t(np.asarray(inputs["dt_self"][sl], f32)),
            "gcs_p": _perm_t(np.asarray(inputs["gc_self"][sl], f32)),
            "bdt_e": _expand_bank(np.asarray(inputs["bank_dt"][sl], f32)),
            "bmsk_e": _expand_bank(
                np.asarray(inputs["bank_mask"][sl]).astype(f32)),
            "bft_p": _perm_bft(np.asarray(inputs["bank_feat"][sl], f32)),
        })
    return ins


_LAST_RESULT = {}


def kernel(**inputs):
    pp = _host_params(inputs["w_time"], inputs["b_time"], inputs["w_node"],
                      inputs["b_node"], inputs["Wq"], inputs["Wk"],
                      inputs["v_att"], inputs["weight"])
    nc = _build_program(pp)
    in_maps = _shard_inputs(inputs)
    import os
    trace = bool(int(os.environ.get("KBENCH_TRACE", "0")))
    res = run_bass_kernel_spmd(nc, in_maps, core_ids=list(range(NCORES)),
                               trace=trace)
    _LAST_RESULT["res"] = res
    outs = [res.results[c]["out"].reshape(EC, 2, H) for c in range(NCORES)]
    return np.ascontiguousarray(np.concatenate(outs, axis=0))

